# revision 55
# baseline (speedup 1.0000x reference)
"""Trainium2 Bass kernel for causal attention with relative-position bias.

Problem (hardcoded): B=16 heads, S=2048, Dh=64, fp32 I/O.
  dots = Q@K^T; bias pos=Q@R_w^T+R_b gathered by sign(j-i)+1; causal mask
  (-1e10 above diag); softmax(dots/sqrt(512)); out = probs@V.

Algebra: within row q the gathered bias is a constant pos0[q] for k<q and
pos1[q] at k==q (k>q masked). Softmax is invariant to per-row constants, so
only the diagonal needs exp((Q[q].(K[q]+R_w[1]-R_w[0]) + R_b[1]-R_b[0])/s).
Logits are small (|z|<=~2.2) so exp runs without max subtraction.

Layout: scores computed transposed, S^T[k,q] (k on partitions):
  S^T = (K^T tile).T @ Q^T          (lhsT=K^T[64,128], rhs=Q^T[64,ncols])
  out^T[d,q] + denominator row = [V|1].T @ exp(S^T)   (accumulated over k)
K^T lives in "fold" layout (even k-tiles on SBUF partitions 0:64, odd on
64:128); Q^T is duplicated on both partition halves.  QK matmuls for
even/odd k-tiles run on disjoint PE row groups (rows 0 / 64): each score
group pairs one even sub-fill (PSUM bank 0) with one odd sub-fill (bank 1,
tile offset 512), so consecutive LDWEIGHTS+MATMUL pairs overlap in the
array (K=64 row-tiling) without ever sharing a PSUM bank.  When the even
sub-range is narrower than 512, the gap is exp'd along with the data (stale
PSUM, never read downstream) - cheaper than an extra ACT call.

The diagonal 128x128 block of each k-tile is zeroed for k>=q by a DVE
multiply with a strictly-upper-triangular 0/1 mask on the exp'd slab.  The
true diagonal term pdiag[q] and its denominator contribution are added in
the epilogue in natural layout:
  out_nat[q,:] += pdiag[q] * [V|1][q,:]   (one precomputed pdv tile per head)

Scheduling: flat software pipeline over all groups of both heads: QK of
group g+2 and PV of group g-1 run while ACT exps group g (one exp call per
group).  PSUM: 3 score bufs (6 banks) + 1 outT accumulator (2 banks) = 8.
Phases run in order [1, 0] so the first groups carry no causal masks and
need only k-tiles 0-1 plus q-tiles 8-15.  Every DMA instruction carries a
multi-us completion-latency tail, so the critical startup tensors (k-tiles
0-15, q-tiles 8-15) are transposed on the PE through fp16 views of PSUM
score slots (the only DMA in their chain is the load itself); junk matmuls
into an unused corner of the staging slot hold the PE HAM clock at 8/8
through the transpose-heavy startup (transposes don't register as PE
activity, which otherwise makes warm-up a per-run coin flip).  Only phase-0
Q (tiles 0-7) and head 1's operands use the xbar fold+unfold path, early
enough that the DMA tails are hidden.  Loads go on the sync HWDGE in
need-order; epilogue xbars + stores share it later.  Diag-block slab masks
run on GpSimd so the exp->mask->PV chain never waits on DVE epilogue work.

Sharding: 16 heads -> 8 NeuronCores, 2 heads/core, no communication.
"""

import sys

if "/opt/trn_rl_repo" not in sys.path:
    sys.path.insert(0, "/opt/trn_rl_repo")

import numpy as np

import concourse.bacc as bacc
import concourse.mybir as mybir
import concourse.tile as tile
from concourse.bass_utils import run_bass_kernel_spmd
from concourse.masks import make_identity, make_upper_triangular

B, S, DH = 16, 2048, 64
N_CORES = 8
HPC = B // N_CORES  # heads per core
P = 128
NT = S // P  # 16 q/k tiles per head
VW = 66  # V row width in SBUF: 64 values + ones col + pad (66*2B keeps 4B align)
OW = 80  # out^T rows padded to xbar multiple of 16 (64 vals + denom + 15 pad)
PH = 1024  # output phase width (outT accumulator cols)
GW = 1024  # score-group tile width
INV_SCALE = float(1.0 / np.sqrt(np.float32(512.0)))

f16 = mybir.dt.float16
f32 = mybir.dt.float32

PHASE_ORDER = (1, 0)


def build_schedule():
    """Merged per-head group list.  Each group pairs sub-ranges of one even
    and one odd k-tile fill, [(ki, qstart, n, tile_off), ...]: even at tile
    offset 0 (<=512 cols, PSUM bank 0), odd at tile offset 512 (bank 1).
    Phase-0 groups (small, PE-heavy) are interleaved into the phase-1
    stream (1024-wide, ACT-heavy) to balance the engines throughout."""
    per_phase = {}
    for ph in (1, 0):
        lo, hi = ph * PH, (ph + 1) * PH
        groups = []
        for t in range(NT // 2):
            e, o = 2 * t, 2 * t + 1
            be, bo = max(P * e, lo), max(P * o, lo)
            if be >= hi:
                continue
            xs = list(range(be, hi, 512)) + [hi]
            for j in range(len(xs) - 1):
                x0, x1 = xs[j], xs[j + 1]
                g = [(e, x0, x1 - x0, 0)]
                ob = max(bo, x0)
                if ob < x1:
                    g.append((o, ob, x1 - ob, 512))
                groups.append((ph, lo, g))
        per_phase[ph] = groups
    return per_phase[1] + per_phase[0]


def chunks_512(a, b):
    """Split [a, b) at multiples of 512."""
    out = []
    while a < b:
        nxt = min(b, (a // 512 + 1) * 512)
        out.append((a, nxt))
        a = nxt
    return out


def _emit(ctx, tc, q_d, k_d, v_d, rw_d, rb_d, out_d):
    nc = tc.nc
    AF = mybir.ActivationFunctionType

    const = ctx.enter_context(tc.tile_pool(name="const", bufs=1))
    ld = ctx.enter_context(tc.tile_pool(name="ld", bufs=2))
    hp = ctx.enter_context(tc.tile_pool(name="hp", bufs=2))
    slabp = ctx.enter_context(tc.tile_pool(name="slab", bufs=6))
    outp = ctx.enter_context(tc.tile_pool(name="outp", bufs=3))
    psc = ctx.enter_context(tc.tile_pool(name="psc", bufs=3, space="PSUM"))
    pout = ctx.enter_context(tc.tile_pool(name="pout", bufs=1, space="PSUM"))

    NH = NT * DH  # 1024

    # R_w rows 0/1 and R_b[0:2] broadcast to all partitions; first on the
    # SWDGE queue (tiny transfers).
    rbc = const.tile([P, 2 * DH + 2], f32)
    nc.scalar.dma_start(out=rbc[:, 0:DH], in_=rw_d[0:1, :].partition_broadcast(P))
    nc.scalar.dma_start(out=rbc[:, DH : 2 * DH], in_=rw_d[1:2, :].partition_broadcast(P))
    nc.scalar.dma_start(
        out=rbc[:, 2 * DH : 2 * DH + 2], in_=rb_d[None, 0:2].partition_broadcast(P)
    )
    # identity (PE-mode transposes) + strictly-upper-triangular mask; the
    # GpSimd queue is otherwise unused so these are ready by ~8us.
    idm = const.tile([P, P], f16)
    make_identity(nc, idm[:])
    m01 = const.tile([P, P], f16)
    make_upper_triangular(nc, m01[:], val=1.0, diag=False)

    # HAM warm-up fodder: PE-mode transposes don't register as PE activity,
    # so without real matmuls the clock stays at 4/8 into the first groups
    # (and whether it warms in time is run-to-run luck).
    junk = const.tile([P, 512], f16)
    nc.vector.memset(junk[:], 0.0)

    st = {
        "groups": [],
        "seg_started": set(),
        "seg_stop": {},
        "kfold": {},
        "qt": {},
        "v3": {},
        "pdv": {},
        "outTs": {},
    }

    def head_tiles(h):
        q32 = ld.tile([P, NH], f32, tag=f"q32_{h}", bufs=1, name=f"q32_{h}")
        k32 = ld.tile([P, NH], f32, tag=f"k32_{h}", bufs=1, name=f"k32_{h}")
        v32 = ld.tile([P, NH], f32, tag=f"v32_{h}", bufs=1, name=f"v32_{h}")
        qf = hp.tile([P, NH], f16, tag="qf", name=f"qf{h}")
        kf = hp.tile([P, NH], f16, tag="kf", name=f"kf{h}")
        kfold = hp.tile([P, 8 * P], f16, tag="kfold", name=f"kfold{h}")
        qfold = hp.tile([P, 8 * P], f16, tag="qfold", name=f"qfold{h}")
        qt = hp.tile([P, S], f16, tag="qt", name=f"qt{h}")
        vaug = hp.tile([P, NT * VW], f16, tag="vaug", name=f"vaug{h}")
        qdup = hp.tile([P, 8 * P], f16, tag="qdup", name=f"qdup{h}")
        return dict(q32=q32, k32=k32, v32=v32, qf=qf, kf=kf, kfold=kfold,
                    qfold=qfold, qt=qt, vaug=vaug, qdup=qdup)

    def load_qk(T, name, src, h, n0, n1, eng=None):
        """Load q/k/v tiles [n0, n1) of head h into the fp32 staging tile."""
        (eng or nc.sync).dma_start(
            out=T[name][:, n0 * DH : n1 * DH].rearrange("p (n d) -> p n d", d=DH),
            in_=src[h].rearrange("(n p) d -> p n d", p=P)[:, n0:n1, :],
        )

    def cast_piece(T, dst, srcname, m0, m1):
        cs = slice(m0 * P, m1 * P)
        nc.vector.tensor_copy(T[dst][:, cs], T[srcname][:, cs])

    def cast_v(T, n0, n1):
        v3 = T["vaug"][:].rearrange("p (n e) -> p n e", e=VW)
        nc.vector.tensor_copy(
            v3[:, n0:n1, 0:DH],
            T["v32"][:].rearrange("p (n d) -> p n d", d=DH)[:, n0:n1],
        )

    def prep_k_piece(T, h, m0, m1, feng=None):
        """Fold-transpose K fold blocks [m0, m1) (k-tiles 2*m0..2*m1)."""
        cs = slice(m0 * P, m1 * P)
        (feng or nc.sync).dma_start_transpose(
            out=T["kfold"][:, cs].rearrange("p (m r) -> p m r", r=P),
            in_=T["kf"][:, cs],
        )

    def prep_q_piece(T, h, m0, m1, uengs, feng=None):
        """Fold + unfold Q fold blocks [m0, m1) -> qt cols 256*m0.., with
        the Q^T data duplicated on both partition halves."""
        cs = slice(m0 * P, m1 * P)
        ms = slice(m0, m1)
        (feng or nc.sync).dma_start_transpose(
            out=T["qfold"][:, cs].rearrange("p (m r) -> p m r", r=P),
            in_=T["qf"][:, cs],
        )
        qt4 = T["qt"][:].rearrange("d (m j r) -> d m j r", j=2, r=P)
        f3q = T["qfold"][:].rearrange("p (m r) -> p m r", r=P)
        uengs[0].dma_start(out=qt4[0:DH, ms, 0, :], in_=f3q[0:DH, ms])
        uengs[1].dma_start(out=qt4[0:DH, ms, 1, :], in_=f3q[DH:P, ms])
        uengs[2].dma_start(out=qt4[DH:P, ms, 0, :], in_=f3q[0:DH, ms])
        uengs[3].dma_start(out=qt4[DH:P, ms, 1, :], in_=f3q[DH:P, ms])

    def prep_pre(T, h):
        """Diagonal-correction terms: pre[q] = Q[q].(K[q]+rdelta)."""
        if h == 0:
            rd16 = const.tile([P, DH], f16)
            nc.vector.tensor_sub(rd16[:], rbc[:, DH : 2 * DH], rbc[:, 0:DH])
            rbbias = const.tile([P, 1], f32)
            nc.vector.tensor_sub(
                rbbias[:], rbc[:, 2 * DH + 1 : 2 * DH + 2], rbc[:, 2 * DH : 2 * DH + 1]
            )
            nc.vector.tensor_scalar_mul(rbbias[:], rbbias[:], INV_SCALE)
            st["rd16"], st["rbbias"] = rd16, rbbias
        t2 = ld.tile([P, NH], f16, tag="t2", name=f"t2_{h}")
        t2_3 = t2[:].rearrange("p (n d) -> p n d", d=DH)
        nc.vector.tensor_add(
            t2_3,
            T["kf"][:].rearrange("p (n d) -> p n d", d=DH),
            st["rd16"][:, None, :].to_broadcast([P, NT, DH]),
        )
        nc.vector.tensor_mul(t2[:], T["qf"][:], t2[:])
        pre = hp.tile([P, NT], f32, tag="pre", name=f"pre{h}")
        nc.vector.tensor_reduce(
            out=pre[:], in_=t2_3, axis=mybir.AxisListType.X, op=mybir.AluOpType.add
        )
        return pre

    def prep_pdv(h, pre, v3):
        """pdiag = exp(pre/s + rbbias); pdv[q,:] = pdiag[q]*[V|1][q,:]."""
        pdiag = hp.tile([P, NT], f16, tag="pdiag", name=f"pdiag{h}")
        nc.scalar.activation(
            pdiag[:], pre[:], AF.Exp, bias=st["rbbias"][:, 0:1], scale=INV_SCALE
        )
        pdv = hp.tile([P, NT * (DH + 1)], f16, tag="pdv", name=f"pdv{h}")
        pdv3 = pdv[:].rearrange("p (n e) -> p n e", e=DH + 1)
        nc.vector.tensor_mul(
            pdv3,
            v3[:, :, 0 : DH + 1],
            pdiag[:, :, None].to_broadcast([P, NT, DH + 1]),
        )
        return pdv3

    # QK weights for k-tile ki come straight from the fold layout
    def kslice(kfold, ki):
        f3 = kfold[:].rearrange("p (m r) -> p m r", r=P)
        half = (ki % 2) * DH
        return f3[half : half + DH, ki // 2, :]

    def emit_qk(gi):
        G = st["groups"][gi]
        sc = psc.tile([P, GW], f32, tag="sc", name="sc")
        kfold, qt = st["kfold"][G["h"]], st["qt"][G["h"]]
        per_fill = []
        for ki, base, n, off in G["fills"]:
            half = (ki % 2) * DH
            per_fill.append(
                [
                    (ki, half, a, b, base + (a - off))
                    for a, b in chunks_512(off, off + n)
                ]
            )
        mx = max(len(c) for c in per_fill)
        for i in range(mx):
            for chunks in per_fill:
                if i < len(chunks):
                    ki, half, a, b, q0 = chunks[i]
                    nc.tensor.matmul(
                        sc[:, a:b],
                        lhsT=kslice(kfold, ki),
                        rhs=qt[half : half + DH, q0 : q0 + (b - a)],
                        start=True,
                        stop=True,
                    )
        G["sc"] = sc

    def emit_exp(gi):
        G = st["groups"][gi]
        ntot = max(f[3] + f[2] for f in G["fills"])
        slab = slabp.tile([P, GW], f16, tag="slab", name="slab")
        nc.scalar.activation(slab[:, 0:ntot], G["sc"][:, 0:ntot], AF.Exp, scale=INV_SCALE)
        G["slab"] = slab
        # zero the invalid (k>=q) half of any diagonal block, on GpSimd
        # (idle mid-loop) so the exp->mask->PV chain never queues behind
        # DVE epilogue work
        for ki, base, n, off in G["fills"]:
            if base == P * ki:
                eng = nc.gpsimd if ki % 2 == 0 else nc.vector
                eng.tensor_mul(
                    slab[:, off : off + P], slab[:, off : off + P], m01[:]
                )

    def emit_pv(gi):
        G = st["groups"][gi]
        slab, v3 = G["slab"], st["v3"][G["h"]]
        for ki, base, n, off in G["fills"]:
            for g0, g1 in chunks_512(base, base + n):
                key = (G["h"], G["ph"], g0 // 512)
                nc.tensor.matmul(
                    G["outT"][:, g0 - G["lo"] : g1 - G["lo"]],
                    lhsT=v3[:, ki, 0 : DH + 1],
                    rhs=slab[:, off + (g0 - base) : off + (g1 - base)],
                    start=(key not in st["seg_started"]),
                    stop=(st["seg_stop"][key] == (gi, ki, g0)),
                    skip_group_check=True,
                )
                st["seg_started"].add(key)

    def emit_epilogue(h, outT, ph_lo, lo, width, pe_path=False):
        """Drain outT cols [lo, lo+width) -> natural layout -> HBM."""
        npm = width // P
        n0 = lo // P
        outTs = st["outTs"][h]
        nc.vector.tensor_copy(
            outTs[0 : DH + 1, lo - ph_lo : lo - ph_lo + width],
            outT[:, lo - ph_lo : lo - ph_lo + width],
        )
        onat = outp.tile([P, (PH // P) * OW], f16, tag="onat", name="onat")
        onat3 = onat[:].rearrange("p (n e) -> p n e", e=OW)[:, 0:npm]
        if pe_path:
            # kernel tail: transpose back on the (idle) PE instead of the
            # xbar, skipping the xbar DMA's completion-latency tail
            est = psc.tile([P, GW], f32, tag="sc", name="est")
            e16 = est[:].bitcast(f16)
            for c in range(npm):
                nc.tensor.transpose(
                    e16[:, c * OW : (c + 1) * OW],
                    outTs[0:OW, lo - ph_lo + c * P : lo - ph_lo + (c + 1) * P],
                    idm[0:OW, 0:OW],
                )
            nc.vector.tensor_copy(onat3, e16[:, 0 : npm * OW].rearrange(
                "p (n e) -> p n e", e=OW))
        else:
            nc.sync.dma_start_transpose(
                out=onat3, in_=outTs[:, lo - ph_lo : lo - ph_lo + width]
            )
        onc = outp.tile([P, (PH // P) * (DH + 1)], f16, tag="onc", name="onc")
        onc3 = onc[:].rearrange("p (n e) -> p n e", e=DH + 1)[:, 0:npm]
        nc.vector.tensor_add(
            onc3, onat3[:, :, 0 : DH + 1], st["pdv"][h][:, n0 : n0 + npm, :]
        )
        recip = outp.tile([P, PH // P], f32, tag="recip", name="recip")
        nc.vector.reciprocal(recip[:, 0:npm, None], onc3[:, :, DH : DH + 1])
        ofin = outp.tile([P, (PH // P) * DH], f32, tag="ofin", name="ofin")
        ofin3 = ofin[:].rearrange("p (n d) -> p n d", d=DH)[:, 0:npm]
        nc.vector.tensor_mul(
            ofin3,
            onc3[:, :, 0:DH],
            recip[:, 0:npm, None].to_broadcast([P, npm, DH]),
        )
        nc.sync.dma_start(
            out=out_d[h].rearrange("(n p) d -> p n d", p=P)[:, n0 : n0 + npm, :],
            in_=ofin3,
        )

    # build the flat group schedule across heads+phases -------------------
    merged = build_schedule()
    for h in range(HPC):
        for ph, lo, g in merged:
            st["groups"].append({"h": h, "ph": ph, "lo": lo, "fills": g})
    for gi, G in enumerate(st["groups"]):
        for ki, base, n, off in G["fills"]:
            for g0, g1 in chunks_512(base, base + n):
                st["seg_stop"][(G["h"], G["ph"], g0 // 512)] = (gi, ki, g0)

    NG = len(st["groups"])
    ph_last = {}  # (h, ph) -> last group index of that phase
    for gi, G in enumerate(st["groups"]):
        ph_last[(G["h"], G["ph"])] = gi
    seg_done_at = {k: v[0] for k, v in st["seg_stop"].items()}

    cur_outT = {}

    def get_outT(G):
        key = (G["h"], G["ph"])
        if key not in cur_outT:
            cur_outT[key] = pout.tile([DH + 1, PH], f32, tag="outT", name="outT")
        return cur_outT[key]

    def emit_pv_and_epi(gi):
        emit_pv(gi)
        G = st["groups"][gi]
        h, ph = G["h"], G["ph"]
        if h == HPC - 1 and ph == 0:
            # final phase: drain per 512-col segment to shorten the tail
            for s in range(PH // 512):
                key = (h, ph, (G["lo"] + 512 * s) // 512)
                if seg_done_at[key] == gi:
                    emit_epilogue(h, G["outT"], G["lo"], G["lo"] + 512 * s, 512,
                                  pe_path=(s == 1))
        elif gi == ph_last[(h, ph)]:
            emit_epilogue(h, G["outT"], G["lo"], G["lo"], PH)

    # ---- startup: phase 1 first.  The first groups need k-tiles 0-1 and
    # q-tiles 8-15: those load first and transpose via the PE (the only DMA
    # in the chain is the load itself, so no DMA completion-latency tails
    # stack up).  Everything else takes the xbar fold path with time to
    # spare.
    T0 = head_tiles(0)
    T1 = head_tiles(1)
    # first two critical loads ride separate HWDGE queues so their DMA
    # completion tails overlap instead of stacking
    load_qk(T0, "k32", k_d, 0, 0, 2, eng=nc.scalar)
    load_qk(T0, "q32", q_d, 0, 8, 12)
    load_qk(T0, "q32", q_d, 0, 12, 16)
    cast_piece(T0, "kf", "k32", 0, 1)
    # duplicate each critical q-tile's 64 d-cols onto both halves of a
    # 128-col block; its PE transpose then lands [d|d-dup, seq] directly
    qd4 = T0["qdup"][:].rearrange("p (n c d) -> p n c d", c=2, d=DH)
    q32_3 = T0["q32"][:].rearrange("p (n d) -> p n d", d=DH)
    stage = psc.tile([P, GW], f32, tag="sc", name="stage")
    st16 = stage[:].bitcast(f16)  # [128, 2048] fp16 view
    # warm-up matmuls into the unused last quarter of the stage tile
    for _ in range(14):
        nc.tensor.matmul(
            stage[:, 768:1024], lhsT=junk[:, 0:P], rhs=junk[:, 0:256],
            start=True, stop=True, skip_group_check=True,
        )
    nc.tensor.transpose(st16[:, 0:P], T0["kf"][:, 0:P], idm[:])
    nc.vector.tensor_copy(T0["kfold"][:, 0:P], st16[:, 0:P])
    for c in range(2):
        ns = slice(8 + 4 * c, 12 + 4 * c)
        nc.vector.tensor_copy(
            qd4[:, 4 * c : 4 * c + 4],
            q32_3[:, ns, None, :].to_broadcast([P, 4, 2, DH]),
        )
        for n in range(4 * c, 4 * c + 4):
            nc.tensor.transpose(
                st16[:, (n + 1) * P : (n + 2) * P],
                T0["qdup"][:, n * P : (n + 1) * P],
                idm[:],
            )
            nc.tensor.matmul(
                stage[:, 768:1024], lhsT=junk[:, 0:P], rhs=junk[:, 0:256],
                start=True, stop=True, skip_group_check=True,
            )
        nc.vector.tensor_copy(
            T0["qt"][:, PH + 512 * c : PH + 512 * (c + 1)],
            st16[:, (1 + 4 * c) * P : (5 + 4 * c) * P],
        )
    # the rest of head 0 via the xbar path
    load_qk(T0, "k32", k_d, 0, 2, 8)
    load_qk(T0, "q32", q_d, 0, 0, 8)
    load_qk(T0, "v32", v_d, 0, 0, 8)
    load_qk(T0, "k32", k_d, 0, 8, 16)
    load_qk(T0, "v32", v_d, 0, 8, 16)
    cast_piece(T0, "kf", "k32", 1, 4)
    for m in range(1, 4):
        nc.tensor.transpose(
            st16[:, (8 + m) * P : (9 + m) * P],
            T0["kf"][:, m * P : (m + 1) * P],
            idm[:],
        )
    nc.vector.tensor_copy(T0["kfold"][:, P : 4 * P], st16[:, 9 * P : 12 * P])
    cast_piece(T0, "kf", "k32", 4, 8)
    cast_piece(T0, "qf", "q32", 0, 4)
    prep_q_piece(T0, 0, 0, 4, [nc.sync] * 4)
    v30 = T0["vaug"][:].rearrange("p (n e) -> p n e", e=VW)
    nc.vector.memset(v30[:, :, DH : DH + 1], 1.0)
    cast_v(T0, 0, 8)
    cast_v(T0, 8, 16)
    cast_piece(T0, "qf", "q32", 4, 8)  # fp16 q-tiles 8-15 for prep_pre
    st["kfold"][0], st["qt"][0], st["v3"][0] = T0["kfold"], T0["qt"], v30
    pre0 = prep_pre(T0, 0)
    outTs0 = outp.tile([OW, PH], f16, tag="outTs", name="outTs0")
    nc.vector.memset(outTs0[DH : OW, :], 0.0)
    st["outTs"][0] = outTs0

    # ---- flat pipeline: ACT exps group g while PE runs QK(g+2) + PV(g-1) ----
    st["groups"][0]["outT"] = get_outT(st["groups"][0])
    emit_qk(0)
    emit_qk(1)

    for gi in range(NG):
        G = st["groups"][gi]
        G["outT"] = get_outT(G)
        emit_exp(gi)
        if gi + 2 < NG:
            emit_qk(gi + 2)
        if gi > 0:
            emit_pv_and_epi(gi - 1)

        # deferred prep work, interleaved into the pipeline (after the
        # epilogue emission so head 1's folds queue behind phase 1's
        # epilogue xbar on the sync queue, not ahead of it)
        if gi == 3:
            # k-tiles 8-15 transpose through a second PE stage (a psc slot
            # that is long free by now), dodging the xbar fold's DMA
            # completion-latency tail
            stage2 = psc.tile([P, GW], f32, tag="sc", name="stage2")
            s216 = stage2[:].bitcast(f16)
            for m in range(4):
                nc.tensor.transpose(
                    s216[:, m * P : (m + 1) * P],
                    T0["kf"][:, (4 + m) * P : (5 + m) * P],
                    idm[:],
                )
            nc.vector.tensor_copy(T0["kfold"][:, 4 * P : 8 * P], s216[:, 0 : 4 * P])
        if gi == 4:
            for n0 in (0, 8):
                load_qk(T1, "k32", k_d, 1, n0, n0 + 8)
                load_qk(T1, "q32", q_d, 1, n0, n0 + 8)
                load_qk(T1, "v32", v_d, 1, n0, n0 + 8)
        if gi == 5:
            st["pdv"][0] = prep_pdv(0, pre0, v30)
        if gi == 6:
            cast_piece(T1, "kf", "k32", 0, 4)
            qd41 = T1["qdup"][:].rearrange("p (n c d) -> p n c d", c=2, d=DH)
            q32_31 = T1["q32"][:].rearrange("p (n d) -> p n d", d=DH)
            nc.vector.tensor_copy(
                qd41[:, :, :, :],
                q32_31[:, 8:16, None, :].to_broadcast([P, 8, 2, DH]),
            )
            cast_piece(T1, "qf", "q32", 4, 8)
            cast_piece(T1, "kf", "k32", 4, 8)
            prep_k_piece(T1, 1, 4, 8)
            cast_piece(T1, "qf", "q32", 0, 4)
            prep_q_piece(T1, 1, 0, 4, [nc.sync] * 4)
            v31 = T1["vaug"][:].rearrange("p (n e) -> p n e", e=VW)
            nc.vector.memset(v31[:, :, DH : DH + 1], 1.0)
            cast_v(T1, 0, 8)
            cast_v(T1, 8, 16)
            st["kfold"][1], st["qt"][1], st["v3"][1] = T1["kfold"], T1["qt"], v31
            st["pre1"] = prep_pre(T1, 1)
            outTs1 = outp.tile([OW, PH], f16, tag="outTs", name="outTs1")
            nc.vector.memset(outTs1[DH : OW, :], 0.0)
            st["outTs"][1] = outTs1
        if gi == 13:
            # head 1's critical transposes ride the PE mid-loop (placed so
            # the PE reaches them only after their casts are data-ready);
            # this skips the xbar fold+unfold DMA completion tails that
            # otherwise stall the head boundary.
            stage3 = psc.tile([P, GW], f32, tag="sc", name="stage3")
            s316 = stage3[:].bitcast(f16)
            for m in range(4):
                nc.tensor.transpose(
                    s316[:, m * P : (m + 1) * P],
                    T1["kf"][:, m * P : (m + 1) * P],
                    idm[:],
                )
            for n in range(8):
                nc.tensor.transpose(
                    s316[:, (4 + n) * P : (5 + n) * P],
                    T1["qdup"][:, n * P : (n + 1) * P],
                    idm[:],
                )
            nc.vector.tensor_copy(T1["kfold"][:, 0 : 4 * P], s316[:, 0 : 4 * P])
            nc.vector.tensor_copy(T1["qt"][:, PH:S], s316[:, 4 * P : 12 * P])
        if gi == 16:
            st["pdv"][1] = prep_pdv(1, st["pre1"], st["v3"][1])

    emit_pv_and_epi(NG - 1)


def build_nc(debug=False):
    from contextlib import ExitStack

    nc = bacc.Bacc("TRN2", target_bir_lowering=False, debug=debug, num_devices=N_CORES)
    q_d = nc.dram_tensor("query", [HPC, S, DH], f32, kind="ExternalInput").ap()
    k_d = nc.dram_tensor("key", [HPC, S, DH], f32, kind="ExternalInput").ap()
    v_d = nc.dram_tensor("value", [HPC, S, DH], f32, kind="ExternalInput").ap()
    rw_d = nc.dram_tensor("R_w", [3, DH], f32, kind="ExternalInput").ap()
    rb_d = nc.dram_tensor("R_b", [3], f32, kind="ExternalInput").ap()
    out_d = nc.dram_tensor("out", [HPC, S, DH], f32, kind="ExternalOutput").ap()
    with tile.TileContext(nc) as tc, ExitStack() as ctx:
        _emit(ctx, tc, q_d, k_d, v_d, rw_d, rb_d, out_d)
    nc.finalize()
    return nc


_NC_CACHE = {}


def _get_nc():
    if "nc" not in _NC_CACHE:
        _NC_CACHE["nc"] = build_nc()
    return _NC_CACHE["nc"]


def kernel(query, key, value, R_w, R_b, trace=False):
    query = np.ascontiguousarray(np.asarray(query, dtype=np.float32))
    key = np.ascontiguousarray(np.asarray(key, dtype=np.float32))
    value = np.ascontiguousarray(np.asarray(value, dtype=np.float32))
    R_w = np.ascontiguousarray(np.asarray(R_w, dtype=np.float32))
    R_b = np.ascontiguousarray(np.asarray(R_b, dtype=np.float32))

    nc = _get_nc()
    in_maps = [
        {
            "query": query[c * HPC : (c + 1) * HPC],
            "key": key[c * HPC : (c + 1) * HPC],
            "value": value[c * HPC : (c + 1) * HPC],
            "R_w": R_w,
            "R_b": R_b,
        }
        for c in range(N_CORES)
    ]
    res = run_bass_kernel_spmd(nc, in_maps, core_ids=list(range(N_CORES)), trace=trace)
    out = np.concatenate([res.results[c]["out"] for c in range(N_CORES)], axis=0)
    if trace:
        kernel.last_results = res
    return out.astype(np.float32, copy=False)


# revision 56
# speedup vs baseline: 1.0084x; 1.0084x over previous
"""Trainium2 Bass kernel for causal attention with relative-position bias.

Problem (hardcoded): B=16 heads, S=2048, Dh=64, fp32 I/O.
  dots = Q@K^T; bias pos=Q@R_w^T+R_b gathered by sign(j-i)+1; causal mask
  (-1e10 above diag); softmax(dots/sqrt(512)); out = probs@V.

Algebra: within row q the gathered bias is a constant pos0[q] for k<q and
pos1[q] at k==q (k>q masked). Softmax is invariant to per-row constants, so
only the diagonal needs exp((Q[q].(K[q]+R_w[1]-R_w[0]) + R_b[1]-R_b[0])/s).
Logits are small (|z|<=~2.2) so exp runs without max subtraction.

Layout: scores computed transposed, S^T[k,q] (k on partitions):
  S^T = (K^T tile).T @ Q^T          (lhsT=K^T[64,128], rhs=Q^T[64,ncols])
  out^T[d,q] + denominator row = [V|1].T @ exp(S^T)   (accumulated over k)
K^T lives in "fold" layout (even k-tiles on SBUF partitions 0:64, odd on
64:128); Q^T is duplicated on both partition halves.  QK matmuls for
even/odd k-tiles run on disjoint PE row groups (rows 0 / 64): each score
group pairs one even sub-fill (PSUM bank 0) with one odd sub-fill (bank 1,
tile offset 512), so consecutive LDWEIGHTS+MATMUL pairs overlap in the
array (K=64 row-tiling) without ever sharing a PSUM bank.  When the even
sub-range is narrower than 512, the gap is exp'd along with the data (stale
PSUM, never read downstream) - cheaper than an extra ACT call.

The diagonal 128x128 block of each k-tile is zeroed for k>=q by a DVE
multiply with a strictly-upper-triangular 0/1 mask on the exp'd slab.  The
true diagonal term pdiag[q] and its denominator contribution are added in
the epilogue in natural layout:
  out_nat[q,:] += pdiag[q] * [V|1][q,:]   (one precomputed pdv tile per head)

Scheduling: flat software pipeline over all groups of both heads: QK of
group g+2 and PV of group g-1 run while ACT exps group g (one exp call per
group).  PSUM: 3 score bufs (6 banks) + 1 outT accumulator (2 banks) = 8.
Phases run in order [1, 0] so the first groups carry no causal masks and
need only k-tiles 0-1 plus q-tiles 8-15.  Every DMA instruction carries a
multi-us completion-latency tail, so the critical startup tensors (k-tiles
0-15, q-tiles 8-15) are transposed on the PE through fp16 views of PSUM
score slots (the only DMA in their chain is the load itself); junk matmuls
into an unused corner of the staging slot hold the PE HAM clock at 8/8
through the transpose-heavy startup (transposes don't register as PE
activity, which otherwise makes warm-up a per-run coin flip).  Only phase-0
Q (tiles 0-7) and head 1's operands use the xbar fold+unfold path, early
enough that the DMA tails are hidden.  Loads go on the sync HWDGE in
need-order; epilogue xbars + stores share it later.  Diag-block slab masks
run on GpSimd so the exp->mask->PV chain never waits on DVE epilogue work.

Sharding: 16 heads -> 8 NeuronCores, 2 heads/core, no communication.
"""

import sys

if "/opt/trn_rl_repo" not in sys.path:
    sys.path.insert(0, "/opt/trn_rl_repo")

import numpy as np

import concourse.bacc as bacc
import concourse.mybir as mybir
import concourse.tile as tile
from concourse.bass_utils import run_bass_kernel_spmd
from concourse.masks import make_identity, make_upper_triangular

B, S, DH = 16, 2048, 64
N_CORES = 8
HPC = B // N_CORES  # heads per core
P = 128
NT = S // P  # 16 q/k tiles per head
VW = 66  # V row width in SBUF: 64 values + ones col + pad (66*2B keeps 4B align)
OW = 80  # out^T rows padded to xbar multiple of 16 (64 vals + denom + 15 pad)
PH = 1024  # output phase width (outT accumulator cols)
GW = 1024  # score-group tile width
INV_SCALE = float(1.0 / np.sqrt(np.float32(512.0)))

f16 = mybir.dt.float16
f32 = mybir.dt.float32

PHASE_ORDER = (1, 0)


def build_schedule():
    """Merged per-head group list.  Each group pairs sub-ranges of one even
    and one odd k-tile fill, [(ki, qstart, n, tile_off), ...]: even at tile
    offset 0 (<=512 cols, PSUM bank 0), odd at tile offset 512 (bank 1).
    Phase-0 groups (small, PE-heavy) are interleaved into the phase-1
    stream (1024-wide, ACT-heavy) to balance the engines throughout."""
    per_phase = {}
    for ph in (1, 0):
        lo, hi = ph * PH, (ph + 1) * PH
        groups = []
        for t in range(NT // 2):
            e, o = 2 * t, 2 * t + 1
            be, bo = max(P * e, lo), max(P * o, lo)
            if be >= hi:
                continue
            xs = list(range(be, hi, 512)) + [hi]
            for j in range(len(xs) - 1):
                x0, x1 = xs[j], xs[j + 1]
                g = [(e, x0, x1 - x0, 0)]
                ob = max(bo, x0)
                if ob < x1:
                    g.append((o, ob, x1 - ob, 512))
                groups.append((ph, lo, g))
        per_phase[ph] = groups
    return per_phase[1] + per_phase[0]


def chunks_512(a, b):
    """Split [a, b) at multiples of 512."""
    out = []
    while a < b:
        nxt = min(b, (a // 512 + 1) * 512)
        out.append((a, nxt))
        a = nxt
    return out


def _emit(ctx, tc, q_d, k_d, v_d, rw_d, rb_d, out_d):
    nc = tc.nc
    AF = mybir.ActivationFunctionType

    const = ctx.enter_context(tc.tile_pool(name="const", bufs=1))
    ld = ctx.enter_context(tc.tile_pool(name="ld", bufs=2))
    hp = ctx.enter_context(tc.tile_pool(name="hp", bufs=2))
    slabp = ctx.enter_context(tc.tile_pool(name="slab", bufs=6))
    outp = ctx.enter_context(tc.tile_pool(name="outp", bufs=3))
    psc = ctx.enter_context(tc.tile_pool(name="psc", bufs=3, space="PSUM"))
    pout = ctx.enter_context(tc.tile_pool(name="pout", bufs=1, space="PSUM"))

    NH = NT * DH  # 1024

    # R_w rows 0/1 and R_b[0:2] broadcast to all partitions; first on the
    # SWDGE queue (tiny transfers).
    rbc = const.tile([P, 2 * DH + 2], f32)
    nc.scalar.dma_start(out=rbc[:, 0:DH], in_=rw_d[0:1, :].partition_broadcast(P))
    nc.scalar.dma_start(out=rbc[:, DH : 2 * DH], in_=rw_d[1:2, :].partition_broadcast(P))
    nc.scalar.dma_start(
        out=rbc[:, 2 * DH : 2 * DH + 2], in_=rb_d[None, 0:2].partition_broadcast(P)
    )
    # identity (PE-mode transposes) + strictly-upper-triangular mask; the
    # GpSimd queue is otherwise unused so these are ready by ~8us.
    idm = const.tile([P, P], f16)
    make_identity(nc, idm[:])
    m01 = const.tile([P, P], f16)
    make_upper_triangular(nc, m01[:], val=1.0, diag=False)

    # HAM warm-up fodder: PE-mode transposes don't register as PE activity,
    # so without real matmuls the clock stays at 4/8 into the first groups
    # (and whether it warms in time is run-to-run luck).
    junk = const.tile([P, 512], f16)
    nc.vector.memset(junk[:], 0.0)

    st = {
        "groups": [],
        "seg_started": set(),
        "seg_stop": {},
        "kfold": {},
        "qt": {},
        "v3": {},
        "pdv": {},
        "outTs": {},
    }

    def head_tiles(h):
        q32 = ld.tile([P, NH], f32, tag=f"q32_{h}", bufs=1, name=f"q32_{h}")
        k32 = ld.tile([P, NH], f32, tag=f"k32_{h}", bufs=1, name=f"k32_{h}")
        v32 = ld.tile([P, NH], f32, tag=f"v32_{h}", bufs=1, name=f"v32_{h}")
        qf = hp.tile([P, NH], f16, tag="qf", name=f"qf{h}")
        kf = hp.tile([P, NH], f16, tag="kf", name=f"kf{h}")
        kfold = hp.tile([P, 8 * P], f16, tag="kfold", name=f"kfold{h}")
        qfold = hp.tile([P, 8 * P], f16, tag="qfold", name=f"qfold{h}")
        qt = hp.tile([P, S], f16, tag="qt", name=f"qt{h}")
        vaug = hp.tile([P, NT * VW], f16, tag="vaug", name=f"vaug{h}")
        qdup = hp.tile([P, 8 * P], f16, tag="qdup", name=f"qdup{h}")
        return dict(q32=q32, k32=k32, v32=v32, qf=qf, kf=kf, kfold=kfold,
                    qfold=qfold, qt=qt, vaug=vaug, qdup=qdup)

    def load_qk(T, name, src, h, n0, n1):
        """Load q/k/v tiles [n0, n1) of head h into the fp32 staging tile."""
        nc.sync.dma_start(
            out=T[name][:, n0 * DH : n1 * DH].rearrange("p (n d) -> p n d", d=DH),
            in_=src[h].rearrange("(n p) d -> p n d", p=P)[:, n0:n1, :],
        )

    def cast_piece(T, dst, srcname, m0, m1):
        cs = slice(m0 * P, m1 * P)
        nc.vector.tensor_copy(T[dst][:, cs], T[srcname][:, cs])

    def cast_v(T, n0, n1):
        v3 = T["vaug"][:].rearrange("p (n e) -> p n e", e=VW)
        nc.vector.tensor_copy(
            v3[:, n0:n1, 0:DH],
            T["v32"][:].rearrange("p (n d) -> p n d", d=DH)[:, n0:n1],
        )

    def prep_k_piece(T, h, m0, m1, feng=None):
        """Fold-transpose K fold blocks [m0, m1) (k-tiles 2*m0..2*m1)."""
        cs = slice(m0 * P, m1 * P)
        (feng or nc.sync).dma_start_transpose(
            out=T["kfold"][:, cs].rearrange("p (m r) -> p m r", r=P),
            in_=T["kf"][:, cs],
        )

    def prep_q_piece(T, h, m0, m1, uengs, feng=None):
        """Fold + unfold Q fold blocks [m0, m1) -> qt cols 256*m0.., with
        the Q^T data duplicated on both partition halves."""
        cs = slice(m0 * P, m1 * P)
        ms = slice(m0, m1)
        (feng or nc.sync).dma_start_transpose(
            out=T["qfold"][:, cs].rearrange("p (m r) -> p m r", r=P),
            in_=T["qf"][:, cs],
        )
        qt4 = T["qt"][:].rearrange("d (m j r) -> d m j r", j=2, r=P)
        f3q = T["qfold"][:].rearrange("p (m r) -> p m r", r=P)
        uengs[0].dma_start(out=qt4[0:DH, ms, 0, :], in_=f3q[0:DH, ms])
        uengs[1].dma_start(out=qt4[0:DH, ms, 1, :], in_=f3q[DH:P, ms])
        uengs[2].dma_start(out=qt4[DH:P, ms, 0, :], in_=f3q[0:DH, ms])
        uengs[3].dma_start(out=qt4[DH:P, ms, 1, :], in_=f3q[DH:P, ms])

    def prep_pre(T, h):
        """Diagonal-correction terms: pre[q] = Q[q].(K[q]+rdelta)."""
        if h == 0:
            rd16 = const.tile([P, DH], f16)
            nc.vector.tensor_sub(rd16[:], rbc[:, DH : 2 * DH], rbc[:, 0:DH])
            rbbias = const.tile([P, 1], f32)
            nc.vector.tensor_sub(
                rbbias[:], rbc[:, 2 * DH + 1 : 2 * DH + 2], rbc[:, 2 * DH : 2 * DH + 1]
            )
            nc.vector.tensor_scalar_mul(rbbias[:], rbbias[:], INV_SCALE)
            st["rd16"], st["rbbias"] = rd16, rbbias
        t2 = ld.tile([P, NH], f16, tag="t2", name=f"t2_{h}")
        t2_3 = t2[:].rearrange("p (n d) -> p n d", d=DH)
        nc.vector.tensor_add(
            t2_3,
            T["kf"][:].rearrange("p (n d) -> p n d", d=DH),
            st["rd16"][:, None, :].to_broadcast([P, NT, DH]),
        )
        nc.vector.tensor_mul(t2[:], T["qf"][:], t2[:])
        pre = hp.tile([P, NT], f32, tag="pre", name=f"pre{h}")
        nc.vector.tensor_reduce(
            out=pre[:], in_=t2_3, axis=mybir.AxisListType.X, op=mybir.AluOpType.add
        )
        return pre

    def prep_pdv(h, pre, v3):
        """pdiag = exp(pre/s + rbbias); pdv[q,:] = pdiag[q]*[V|1][q,:]."""
        pdiag = hp.tile([P, NT], f16, tag="pdiag", name=f"pdiag{h}")
        nc.scalar.activation(
            pdiag[:], pre[:], AF.Exp, bias=st["rbbias"][:, 0:1], scale=INV_SCALE
        )
        pdv = hp.tile([P, NT * (DH + 1)], f16, tag="pdv", name=f"pdv{h}")
        pdv3 = pdv[:].rearrange("p (n e) -> p n e", e=DH + 1)
        nc.vector.tensor_mul(
            pdv3,
            v3[:, :, 0 : DH + 1],
            pdiag[:, :, None].to_broadcast([P, NT, DH + 1]),
        )
        return pdv3

    # QK weights for k-tile ki come straight from the fold layout
    def kslice(kfold, ki):
        f3 = kfold[:].rearrange("p (m r) -> p m r", r=P)
        half = (ki % 2) * DH
        return f3[half : half + DH, ki // 2, :]

    def emit_qk(gi):
        G = st["groups"][gi]
        sc = psc.tile([P, GW], f32, tag="sc", name="sc")
        kfold, qt = st["kfold"][G["h"]], st["qt"][G["h"]]
        per_fill = []
        for ki, base, n, off in G["fills"]:
            half = (ki % 2) * DH
            per_fill.append(
                [
                    (ki, half, a, b, base + (a - off))
                    for a, b in chunks_512(off, off + n)
                ]
            )
        mx = max(len(c) for c in per_fill)
        for i in range(mx):
            for chunks in per_fill:
                if i < len(chunks):
                    ki, half, a, b, q0 = chunks[i]
                    nc.tensor.matmul(
                        sc[:, a:b],
                        lhsT=kslice(kfold, ki),
                        rhs=qt[half : half + DH, q0 : q0 + (b - a)],
                        start=True,
                        stop=True,
                    )
        G["sc"] = sc

    def emit_exp(gi):
        G = st["groups"][gi]
        ntot = max(f[3] + f[2] for f in G["fills"])
        slab = slabp.tile([P, GW], f16, tag="slab", name="slab")
        nc.scalar.activation(slab[:, 0:ntot], G["sc"][:, 0:ntot], AF.Exp, scale=INV_SCALE)
        G["slab"] = slab
        # zero the invalid (k>=q) half of any diagonal block, on GpSimd
        # (idle mid-loop) so the exp->mask->PV chain never queues behind
        # DVE epilogue work
        for ki, base, n, off in G["fills"]:
            if base == P * ki:
                eng = nc.gpsimd if ki % 2 == 0 else nc.vector
                eng.tensor_mul(
                    slab[:, off : off + P], slab[:, off : off + P], m01[:]
                )

    def emit_pv(gi):
        G = st["groups"][gi]
        slab, v3 = G["slab"], st["v3"][G["h"]]
        for ki, base, n, off in G["fills"]:
            for g0, g1 in chunks_512(base, base + n):
                key = (G["h"], G["ph"], g0 // 512)
                nc.tensor.matmul(
                    G["outT"][:, g0 - G["lo"] : g1 - G["lo"]],
                    lhsT=v3[:, ki, 0 : DH + 1],
                    rhs=slab[:, off + (g0 - base) : off + (g1 - base)],
                    start=(key not in st["seg_started"]),
                    stop=(st["seg_stop"][key] == (gi, ki, g0)),
                    skip_group_check=True,
                )
                st["seg_started"].add(key)

    def emit_epilogue(h, outT, ph_lo, lo, width, pe_path=False):
        """Drain outT cols [lo, lo+width) -> natural layout -> HBM."""
        npm = width // P
        n0 = lo // P
        outTs = st["outTs"][h]
        nc.vector.tensor_copy(
            outTs[0 : DH + 1, lo - ph_lo : lo - ph_lo + width],
            outT[:, lo - ph_lo : lo - ph_lo + width],
        )
        onat = outp.tile([P, (PH // P) * OW], f16, tag="onat", name="onat")
        onat3 = onat[:].rearrange("p (n e) -> p n e", e=OW)[:, 0:npm]
        if pe_path:
            # kernel tail: transpose back on the (idle) PE instead of the
            # xbar, skipping the xbar DMA's completion-latency tail
            est = psc.tile([P, GW], f32, tag="sc", name="est")
            e16 = est[:].bitcast(f16)
            for c in range(npm):
                nc.tensor.transpose(
                    e16[:, c * OW : (c + 1) * OW],
                    outTs[0:OW, lo - ph_lo + c * P : lo - ph_lo + (c + 1) * P],
                    idm[0:OW, 0:OW],
                )
            nc.vector.tensor_copy(onat3, e16[:, 0 : npm * OW].rearrange(
                "p (n e) -> p n e", e=OW))
        else:
            nc.sync.dma_start_transpose(
                out=onat3, in_=outTs[:, lo - ph_lo : lo - ph_lo + width]
            )
        onc = outp.tile([P, (PH // P) * (DH + 1)], f16, tag="onc", name="onc")
        onc3 = onc[:].rearrange("p (n e) -> p n e", e=DH + 1)[:, 0:npm]
        nc.vector.tensor_add(
            onc3, onat3[:, :, 0 : DH + 1], st["pdv"][h][:, n0 : n0 + npm, :]
        )
        recip = outp.tile([P, PH // P], f32, tag="recip", name="recip")
        nc.vector.reciprocal(recip[:, 0:npm, None], onc3[:, :, DH : DH + 1])
        ofin = outp.tile([P, (PH // P) * DH], f32, tag="ofin", name="ofin")
        ofin3 = ofin[:].rearrange("p (n d) -> p n d", d=DH)[:, 0:npm]
        nc.vector.tensor_mul(
            ofin3,
            onc3[:, :, 0:DH],
            recip[:, 0:npm, None].to_broadcast([P, npm, DH]),
        )
        nc.sync.dma_start(
            out=out_d[h].rearrange("(n p) d -> p n d", p=P)[:, n0 : n0 + npm, :],
            in_=ofin3,
        )

    # build the flat group schedule across heads+phases -------------------
    merged = build_schedule()
    for h in range(HPC):
        for ph, lo, g in merged:
            st["groups"].append({"h": h, "ph": ph, "lo": lo, "fills": g})
    for gi, G in enumerate(st["groups"]):
        for ki, base, n, off in G["fills"]:
            for g0, g1 in chunks_512(base, base + n):
                st["seg_stop"][(G["h"], G["ph"], g0 // 512)] = (gi, ki, g0)

    NG = len(st["groups"])
    ph_last = {}  # (h, ph) -> last group index of that phase
    for gi, G in enumerate(st["groups"]):
        ph_last[(G["h"], G["ph"])] = gi
    seg_done_at = {k: v[0] for k, v in st["seg_stop"].items()}

    cur_outT = {}

    def get_outT(G):
        key = (G["h"], G["ph"])
        if key not in cur_outT:
            cur_outT[key] = pout.tile([DH + 1, PH], f32, tag="outT", name="outT")
        return cur_outT[key]

    def emit_pv_and_epi(gi):
        emit_pv(gi)
        G = st["groups"][gi]
        h, ph = G["h"], G["ph"]
        if h == HPC - 1 and ph == 0:
            # final phase: drain per 512-col segment to shorten the tail
            for s in range(PH // 512):
                key = (h, ph, (G["lo"] + 512 * s) // 512)
                if seg_done_at[key] == gi:
                    emit_epilogue(h, G["outT"], G["lo"], G["lo"] + 512 * s, 512,
                                  pe_path=(s == 1))
        elif gi == ph_last[(h, ph)]:
            emit_epilogue(h, G["outT"], G["lo"], G["lo"], PH)

    # ---- startup: phase 1 first.  The first groups need k-tiles 0-1 and
    # q-tiles 8-15: those load first and transpose via the PE (the only DMA
    # in the chain is the load itself, so no DMA completion-latency tails
    # stack up).  Everything else takes the xbar fold path with time to
    # spare.
    T0 = head_tiles(0)
    T1 = head_tiles(1)
    load_qk(T0, "k32", k_d, 0, 0, 2)
    load_qk(T0, "q32", q_d, 0, 8, 12)
    load_qk(T0, "q32", q_d, 0, 12, 16)
    cast_piece(T0, "kf", "k32", 0, 1)
    # duplicate each critical q-tile's 64 d-cols onto both halves of a
    # 128-col block; its PE transpose then lands [d|d-dup, seq] directly
    qd4 = T0["qdup"][:].rearrange("p (n c d) -> p n c d", c=2, d=DH)
    q32_3 = T0["q32"][:].rearrange("p (n d) -> p n d", d=DH)
    stage = psc.tile([P, GW], f32, tag="sc", name="stage")
    st16 = stage[:].bitcast(f16)  # [128, 2048] fp16 view
    # warm-up matmuls into the unused last quarter of the stage tile
    for _ in range(14):
        nc.tensor.matmul(
            stage[:, 768:1024], lhsT=junk[:, 0:P], rhs=junk[:, 0:256],
            start=True, stop=True, skip_group_check=True,
        )
    nc.tensor.transpose(st16[:, 0:P], T0["kf"][:, 0:P], idm[:])
    nc.vector.tensor_copy(T0["kfold"][:, 0:P], st16[:, 0:P])
    for c in range(2):
        ns = slice(8 + 4 * c, 12 + 4 * c)
        nc.vector.tensor_copy(
            qd4[:, 4 * c : 4 * c + 4],
            q32_3[:, ns, None, :].to_broadcast([P, 4, 2, DH]),
        )
        for n in range(4 * c, 4 * c + 4):
            nc.tensor.transpose(
                st16[:, (n + 1) * P : (n + 2) * P],
                T0["qdup"][:, n * P : (n + 1) * P],
                idm[:],
            )
            nc.tensor.matmul(
                stage[:, 768:1024], lhsT=junk[:, 0:P], rhs=junk[:, 0:256],
                start=True, stop=True, skip_group_check=True,
            )
        nc.vector.tensor_copy(
            T0["qt"][:, PH + 512 * c : PH + 512 * (c + 1)],
            st16[:, (1 + 4 * c) * P : (5 + 4 * c) * P],
        )
    # the rest of head 0 via the xbar path
    load_qk(T0, "k32", k_d, 0, 2, 8)
    load_qk(T0, "q32", q_d, 0, 0, 8)
    load_qk(T0, "v32", v_d, 0, 0, 8)
    load_qk(T0, "k32", k_d, 0, 8, 16)
    load_qk(T0, "v32", v_d, 0, 8, 16)
    cast_piece(T0, "kf", "k32", 1, 4)
    for m in range(1, 4):
        nc.tensor.transpose(
            st16[:, (8 + m) * P : (9 + m) * P],
            T0["kf"][:, m * P : (m + 1) * P],
            idm[:],
        )
    nc.vector.tensor_copy(T0["kfold"][:, P : 4 * P], st16[:, 9 * P : 12 * P])
    cast_piece(T0, "kf", "k32", 4, 8)
    cast_piece(T0, "qf", "q32", 0, 4)
    prep_q_piece(T0, 0, 0, 4, [nc.sync] * 4)
    v30 = T0["vaug"][:].rearrange("p (n e) -> p n e", e=VW)
    nc.vector.memset(v30[:, :, DH : DH + 1], 1.0)
    cast_v(T0, 0, 8)
    cast_v(T0, 8, 16)
    cast_piece(T0, "qf", "q32", 4, 8)  # fp16 q-tiles 8-15 for prep_pre
    st["kfold"][0], st["qt"][0], st["v3"][0] = T0["kfold"], T0["qt"], v30
    pre0 = prep_pre(T0, 0)
    outTs0 = outp.tile([OW, PH], f16, tag="outTs", name="outTs0")
    nc.vector.memset(outTs0[DH : OW, :], 0.0)
    st["outTs"][0] = outTs0

    # ---- flat pipeline: ACT exps group g while PE runs QK(g+2) + PV(g-1) ----
    st["groups"][0]["outT"] = get_outT(st["groups"][0])
    emit_qk(0)
    emit_qk(1)

    for gi in range(NG):
        G = st["groups"][gi]
        G["outT"] = get_outT(G)
        emit_exp(gi)
        if gi + 2 < NG:
            emit_qk(gi + 2)
        if gi > 0:
            emit_pv_and_epi(gi - 1)

        # deferred prep work, interleaved into the pipeline (after the
        # epilogue emission so head 1's folds queue behind phase 1's
        # epilogue xbar on the sync queue, not ahead of it)
        if gi == 3:
            # k-tiles 8-15 transpose through a second PE stage (a psc slot
            # that is long free by now), dodging the xbar fold's DMA
            # completion-latency tail
            stage2 = psc.tile([P, GW], f32, tag="sc", name="stage2")
            s216 = stage2[:].bitcast(f16)
            for m in range(4):
                nc.tensor.transpose(
                    s216[:, m * P : (m + 1) * P],
                    T0["kf"][:, (4 + m) * P : (5 + m) * P],
                    idm[:],
                )
            nc.vector.tensor_copy(T0["kfold"][:, 4 * P : 8 * P], s216[:, 0 : 4 * P])
        if gi == 4:
            for n0 in (0, 8):
                load_qk(T1, "k32", k_d, 1, n0, n0 + 8)
                load_qk(T1, "q32", q_d, 1, n0, n0 + 8)
                load_qk(T1, "v32", v_d, 1, n0, n0 + 8)
        if gi == 5:
            st["pdv"][0] = prep_pdv(0, pre0, v30)
        if gi == 6:
            cast_piece(T1, "kf", "k32", 0, 4)
            qd41 = T1["qdup"][:].rearrange("p (n c d) -> p n c d", c=2, d=DH)
            q32_31 = T1["q32"][:].rearrange("p (n d) -> p n d", d=DH)
            nc.vector.tensor_copy(
                qd41[:, :, :, :],
                q32_31[:, 8:16, None, :].to_broadcast([P, 8, 2, DH]),
            )
            cast_piece(T1, "qf", "q32", 4, 8)
            cast_piece(T1, "kf", "k32", 4, 8)
            prep_k_piece(T1, 1, 4, 8)
            cast_piece(T1, "qf", "q32", 0, 4)
            prep_q_piece(T1, 1, 0, 4, [nc.sync] * 4)
            v31 = T1["vaug"][:].rearrange("p (n e) -> p n e", e=VW)
            nc.vector.memset(v31[:, :, DH : DH + 1], 1.0)
            cast_v(T1, 0, 8)
            cast_v(T1, 8, 16)
            st["kfold"][1], st["qt"][1], st["v3"][1] = T1["kfold"], T1["qt"], v31
            st["pre1"] = prep_pre(T1, 1)
            outTs1 = outp.tile([OW, PH], f16, tag="outTs", name="outTs1")
            nc.vector.memset(outTs1[DH : OW, :], 0.0)
            st["outTs"][1] = outTs1
        if gi == 13:
            # head 1's critical transposes ride the PE mid-loop (placed so
            # the PE reaches them only after their casts are data-ready);
            # this skips the xbar fold+unfold DMA completion tails that
            # otherwise stall the head boundary.
            stage3 = psc.tile([P, GW], f32, tag="sc", name="stage3")
            s316 = stage3[:].bitcast(f16)
            for m in range(4):
                nc.tensor.transpose(
                    s316[:, m * P : (m + 1) * P],
                    T1["kf"][:, m * P : (m + 1) * P],
                    idm[:],
                )
            for n in range(8):
                nc.tensor.transpose(
                    s316[:, (4 + n) * P : (5 + n) * P],
                    T1["qdup"][:, n * P : (n + 1) * P],
                    idm[:],
                )
            nc.vector.tensor_copy(T1["kfold"][:, 0 : 4 * P], s316[:, 0 : 4 * P])
            nc.vector.tensor_copy(T1["qt"][:, PH:S], s316[:, 4 * P : 12 * P])
        if gi == 16:
            st["pdv"][1] = prep_pdv(1, st["pre1"], st["v3"][1])

    emit_pv_and_epi(NG - 1)


def build_nc(debug=False):
    from contextlib import ExitStack

    nc = bacc.Bacc("TRN2", target_bir_lowering=False, debug=debug, num_devices=N_CORES)
    q_d = nc.dram_tensor("query", [HPC, S, DH], f32, kind="ExternalInput").ap()
    k_d = nc.dram_tensor("key", [HPC, S, DH], f32, kind="ExternalInput").ap()
    v_d = nc.dram_tensor("value", [HPC, S, DH], f32, kind="ExternalInput").ap()
    rw_d = nc.dram_tensor("R_w", [3, DH], f32, kind="ExternalInput").ap()
    rb_d = nc.dram_tensor("R_b", [3], f32, kind="ExternalInput").ap()
    out_d = nc.dram_tensor("out", [HPC, S, DH], f32, kind="ExternalOutput").ap()
    with tile.TileContext(nc) as tc, ExitStack() as ctx:
        _emit(ctx, tc, q_d, k_d, v_d, rw_d, rb_d, out_d)
    nc.finalize()
    return nc


_NC_CACHE = {}


def _get_nc():
    if "nc" not in _NC_CACHE:
        _NC_CACHE["nc"] = build_nc()
    return _NC_CACHE["nc"]


def kernel(query, key, value, R_w, R_b, trace=False):
    query = np.ascontiguousarray(np.asarray(query, dtype=np.float32))
    key = np.ascontiguousarray(np.asarray(key, dtype=np.float32))
    value = np.ascontiguousarray(np.asarray(value, dtype=np.float32))
    R_w = np.ascontiguousarray(np.asarray(R_w, dtype=np.float32))
    R_b = np.ascontiguousarray(np.asarray(R_b, dtype=np.float32))

    nc = _get_nc()
    in_maps = [
        {
            "query": query[c * HPC : (c + 1) * HPC],
            "key": key[c * HPC : (c + 1) * HPC],
            "value": value[c * HPC : (c + 1) * HPC],
            "R_w": R_w,
            "R_b": R_b,
        }
        for c in range(N_CORES)
    ]
    res = run_bass_kernel_spmd(nc, in_maps, core_ids=list(range(N_CORES)), trace=trace)
    out = np.concatenate([res.results[c]["out"] for c in range(N_CORES)], axis=0)
    if trace:
        kernel.last_results = res
    return out.astype(np.float32, copy=False)
